# revision 2
# baseline (speedup 1.0000x reference)
"""Multi-head attention (B=2, T=2048, C=1024, H=16, hd=64, RoPE, full mask)
on 8 TRN2 NeuronCores.

Sharding: tensor-parallel over (batch, head-group). Core c handles batch
c//4 and heads [4*(c%4) .. 4*(c%4)+3]. Host sums the 4 partial output
projections per batch.

Math identical to the f16 baseline (bf16/f16 attention core, f32 PSUM).
The speedup over the baseline is scheduling: the attention phase is
ACT-bound (128 exps x ~1.1us), so the projection/RoPE/v-projection
prologue is interleaved INTO the attention loop's PE slack via an
explicit feed queue, instead of running serially before it (which left
the scalar engine idle for 80us). Pre-roll is only: k-projection,
q(block 0) projection, and the first half of the v-projection.
"""

import ml_dtypes
import numpy as np

import concourse.bacc as bacc
import concourse.mybir as mybir
import concourse.tile as tile
from concourse.bass_utils import run_bass_kernel_spmd

B, T, C = 2, 2048, 1024
N_HEAD = 16
HD = 64
N_CORES = 8
HPC = 4
GC = HPC * HD  # 256

P = 128
KC = C // P  # 8
NQB = 4
TQ = T // NQB  # 512
NKB = T // P  # 16
VW = HD + 2  # 66

F32 = mybir.dt.float32
F32R = mybir.dt.float32r
F16 = mybir.dt.float16

_PROGRAM = None


def _build_program():
    nc = bacc.Bacc(
        "TRN2", target_bir_lowering=False, debug=False, num_devices=N_CORES
    )

    xT_d = nc.dram_tensor("xT", [C, T], F16, kind="ExternalInput").ap()
    wqkT_d = nc.dram_tensor("wqkT", [C, 4 * P], F16, kind="ExternalInput").ap()
    wvT_d = nc.dram_tensor("wvT", [C, GC], F16, kind="ExternalInput").ap()
    wpT_d = nc.dram_tensor("wpT", [GC, C], F16, kind="ExternalInput").ap()
    cc_d = nc.dram_tensor("cc", [P, T], F16, kind="ExternalInput").ap()
    ss_d = nc.dram_tensor("ss", [P, T], F16, kind="ExternalInput").ap()
    psw_d = nc.dram_tensor("psw", [P, P], F16, kind="ExternalInput").ap()
    emat_d = nc.dram_tensor("emat", [HPC, 2 * P], F32R, kind="ExternalInput").ap()
    ident_d = nc.dram_tensor("ident", [P, P], F32, kind="ExternalInput").ap()
    esel_d = nc.dram_tensor("esel", [P, HPC], F32, kind="ExternalInput").ap()
    y_d = nc.dram_tensor("y", [T, C], F32, kind="ExternalOutput").ap()

    with tile.TileContext(nc) as tc:
        with (
            tc.tile_pool(name="consts", bufs=1) as consts,
            tc.tile_pool(name="bigs", bufs=1) as bigs,
            tc.tile_pool(name="tmps", bufs=2) as tmps,
            tc.tile_pool(name="expool", bufs=3) as expool,
            tc.tile_pool(name="psA", bufs=2, space="PSUM") as psA,
            tc.tile_pool(name="psB", bufs=2, space="PSUM") as psB,
            tc.tile_pool(name="psC", bufs=2, space="PSUM") as psC,
        ):
            # ---- resident loads ----
            x_k = []
            for kc in range(KC):
                t = bigs.tile([P, T], F16, tag=f"x{kc}", name=f"x{kc}")
                nc.sync.dma_start(out=t, in_=xT_d[kc * P : (kc + 1) * P, :])
                x_k.append(t)
            wqk_k = []
            for kc in range(KC):
                t = bigs.tile([P, 4 * P], F16, tag=f"wqk{kc}", name=f"wqk{kc}")
                nc.sync.dma_start(out=t, in_=wqkT_d[kc * P : (kc + 1) * P, :])
                wqk_k.append(t)
            wv_k = []
            for kc in range(KC):
                t = bigs.tile([P, GC], F16, tag=f"wv{kc}", name=f"wv{kc}")
                nc.sync.dma_start(out=t, in_=wvT_d[kc * P : (kc + 1) * P, :])
                wv_k.append(t)
            wp_k = []
            for kb in range(2):
                t = bigs.tile([P, C], F16, tag=f"wp{kb}", name=f"wp{kb}")
                nc.sync.dma_start(out=t, in_=wpT_d[kb * P : (kb + 1) * P, :])
                wp_k.append(t)
            cc_t = consts.tile([P, T], F16, tag="cc")
            nc.sync.dma_start(out=cc_t, in_=cc_d)
            ss_t = consts.tile([P, T], F16, tag="ss")
            nc.sync.dma_start(out=ss_t, in_=ss_d)
            psw_t = consts.tile([P, P], F16, tag="psw")
            nc.sync.dma_start(out=psw_t, in_=psw_d)
            emat_t = consts.tile([HPC, 2 * P], F32R, tag="emat")
            nc.sync.dma_start(out=emat_t, in_=emat_d)
            ident_t = consts.tile([P, P], F32, tag="ident")
            nc.sync.dma_start(out=ident_t, in_=ident_d)
            esel_t = consts.tile([P, HPC], F32, tag="esel")
            nc.sync.dma_start(out=esel_t, in_=esel_d)
            ones_f = consts.tile([P, 2 * HPC], F32, tag="ones_f")
            nc.vector.memset(ones_f, 1.0)

            # ---- shared state ----
            qk_sb = [
                bigs.tile([P, T], F16, tag=f"qk{mb}", name=f"qk{mb}")
                for mb in range(4)
            ]
            va_list = [
                bigs.tile([P, HPC * VW], F16, tag=f"va{tb}", name=f"va{tb}")
                for tb in range(NKB)
            ]

            # ---- prologue building blocks (emitted piecewise) ----
            # pre-roll chunks use psA; feed-time chunks use psB tiles so the
            # attention loop's st2 double-buffer rotation is never blocked.
            proj_ps = {}

            def proj_mms(mb, n, kc0, kcn, aux):
                """projection matmuls for chunks kc0..kc0+kcn-1"""
                ns = slice(n * TQ, (n + 1) * TQ)
                if (mb, n) not in proj_ps:
                    if aux:
                        main = psB.tile([P, TQ], F32, tag="aux", name=f"pp{mb}_{n}")
                        proj_ps[(mb, n)] = (main, None)
                    else:
                        t = psA.tile(
                            [P, 2 * TQ], F32, tag="mmps", name=f"pp{mb}_{n}"
                        )
                        proj_ps[(mb, n)] = (t[:, 0:TQ], t[:, TQ : 2 * TQ])
                ps = proj_ps[(mb, n)][0]
                for kc in range(kc0, kc0 + kcn):
                    nc.tensor.matmul(
                        ps,
                        lhsT=wqk_k[kc][:, mb * P : (mb + 1) * P],
                        rhs=x_k[kc][:, ns],
                        start=(kc == 0),
                        stop=(kc == KC - 1),
                    )

            def proj_finish(mb, n, aux):
                """copy out, psw swap matmul, RoPE combine -> qk_sb[mb]"""
                ns = slice(n * TQ, (n + 1) * TQ)
                pss, sw = proj_ps.pop((mb, n))
                sb = qk_sb[mb]
                nc.vector.tensor_copy(sb[:, ns], pss)
                if sw is None:
                    sw = psB.tile([P, TQ], F32, tag="aux", name=f"sw{mb}_{n}")
                nc.tensor.matmul(
                    sw, lhsT=psw_t, rhs=sb[:, ns], start=True, stop=True
                )
                nc.vector.tensor_mul(sb[:, ns], sb[:, ns], cc_t[:, ns])
                tmp = tmps.tile([P, TQ], F16, tag="ropetmp", name=f"rt{mb}_{n}")
                nc.vector.tensor_mul(tmp, sw, ss_t[:, ns])
                nc.vector.tensor_add(sb[:, ns], sb[:, ns], tmp)

            def vproj_all(tb, aux):
                if aux:
                    vps = psB.tile([P, TQ], F32, tag="aux", name=f"vp{tb}")[:, 0:GC]
                else:
                    vps = psA.tile(
                        [P, 2 * TQ], F32, tag="mmps", name=f"vp{tb}"
                    )[:, 0:GC]
                for kc in range(KC):
                    nc.tensor.matmul(
                        vps,
                        lhsT=x_k[kc][:, tb * P : (tb + 1) * P],
                        rhs=wv_k[kc],
                        start=(kc == 0),
                        stop=(kc == KC - 1),
                    )
                va = va_list[tb]
                va4 = va.rearrange("p (h c) -> p h c", c=VW)
                nc.vector.tensor_copy(
                    va4[:, :, HD : HD + 2],
                    ones_f.rearrange("p (h c) -> p h c", c=2),
                )
                nc.vector.tensor_copy(
                    va4[:, :, 0:HD], vps.rearrange("p (h c) -> p h c", c=HD)
                )

            # ---- feed queue with need-by draining ----
            feed = []
            done_keys = set()

            def pop_feed():
                if feed:
                    key, fn = feed.pop(0)
                    fn()
                    done_keys.add(key)

            def drain_until(key):
                while feed and key not in done_keys:
                    pop_feed()

            pend = {}
            qb_state = {}

            def attention_pair(qb, p):
                qs = slice(qb * TQ, (qb + 1) * TQ)
                if qb not in qb_state:
                    oevp = [
                        tmps.tile(
                            [P, TQ], F32, tag=f"oevp{i}", name=f"oevp{i}_{qb}",
                            bufs=2,
                        )
                        for i in range(2)
                    ]
                    den4 = tmps.tile(
                        [P, TQ], F32, tag="den4", name=f"den4_{qb}", bufs=2
                    )
                    nc.vector.memset(den4, 1.0)
                    qb_state[qb] = (oevp, den4)
                oevp, den4 = qb_state[qb]
                qt = qk_sb[p]
                kt = qk_sb[2 + p]
                oau = [
                    psC.tile([VW, TQ], F32, tag="oau", name=f"oau{qb}{p}{i}")
                    for i in range(2)
                ]
                exs = {}
                for kb in range(NKB + 1):
                    if kb < NKB:
                        pop_feed()
                        st2 = psA.tile(
                            [P, 2 * TQ], F32, tag="mmps", name=f"s{qb}{p}{kb}"
                        )
                        ks = slice(kb * P, (kb + 1) * P)
                        for i in range(2):
                            nc.tensor.matmul(
                                st2[:, i * TQ : (i + 1) * TQ],
                                lhsT=kt[i * HD : (i + 1) * HD, ks],
                                rhs=qt[i * HD : (i + 1) * HD, qs],
                                start=True,
                                stop=True,
                            )
                        ex = expool.tile([P, 2 * TQ], F16, tag="ex", name="ex")
                        nc.scalar.activation(
                            out=ex,
                            in_=st2,
                            func=mybir.ActivationFunctionType.Exp,
                            scale=1.0 / np.sqrt(HD),
                        )
                        exs[kb] = ex
                    if kb >= 1:
                        pk = kb - 1
                        exp_prev = exs.pop(pk)
                        for i in range(2):
                            nc.tensor.matmul(
                                oau[i],
                                lhsT=va_list[pk][:, (2 * p + i) * VW :][:, :VW],
                                rhs=exp_prev[:, i * TQ : (i + 1) * TQ],
                                start=(pk == 0),
                                stop=(pk == NKB - 1),
                            )
                # stage o (unnormalized) and denominators
                for i in range(2):
                    nc.vector.tensor_copy(
                        oevp[p][i * HD : (i + 1) * HD, :], oau[i][0:HD, :]
                    )
                    r = 32 * (2 * p + i)
                    nc.vector.tensor_copy(
                        den4[r : r + 1, :], oau[i][HD : HD + 1, :]
                    )
                if p == 1:
                    pend[qb] = qb_state.pop(qb)

            def finalize(qb):
                oevp, den4 = pend.pop(qb)
                denT = psB.tile([P, 4 * HPC], F32, tag="aux", name=f"dT{qb}")
                for c in range(4):
                    nc.tensor.matmul(
                        denT[:, c * HPC : (c + 1) * HPC],
                        lhsT=den4[:, c * P : (c + 1) * P],
                        rhs=esel_t,
                        start=True,
                        stop=True,
                    )
                rdenT = tmps.tile([P, 4 * HPC], F32, tag="rdenT", name=f"rdT{qb}")
                nc.vector.reciprocal(rdenT, denT)
                rden_ps = psB.tile([HPC, TQ], F32, tag="aux", name=f"rp{qb}")
                for c in range(4):
                    nc.tensor.transpose(
                        rden_ps[:, c * P : (c + 1) * P],
                        rdenT[:, c * HPC : (c + 1) * HPC],
                        ident_t,
                    )
                rden4 = tmps.tile([HPC, TQ], F32R, tag="rden4", name=f"rd4{qb}")
                with nc.allow_low_precision(reason="f32r round of 1/den"):
                    nc.vector.tensor_copy(rden4, rden_ps)
                o_sb = [
                    tmps.tile(
                        [P, TQ], F16, tag=f"osb{i}", name=f"osb{i}_{qb}", bufs=2
                    )
                    for i in range(2)
                ]
                for i in range(2):
                    bc = psB.tile([P, TQ], F32, tag="aux", name=f"bc{qb}{i}")
                    nc.tensor.matmul(
                        bc,
                        lhsT=emat_t[:, i * P : (i + 1) * P],
                        rhs=rden4,
                        start=True,
                        stop=True,
                    )
                    nc.vector.tensor_mul(o_sb[i], oevp[i], bc)

                for tch in range(TQ // P):
                    for cch in range(C // TQ):
                        yp = psB.tile(
                            [P, TQ], F32, tag="aux", name=f"yp{qb}{tch}{cch}"
                        )
                        for kb in range(2):
                            nc.tensor.matmul(
                                yp,
                                lhsT=o_sb[kb][:, tch * P : (tch + 1) * P],
                                rhs=wp_k[kb][:, cch * TQ : (cch + 1) * TQ],
                                start=(kb == 0),
                                stop=(kb == 1),
                            )
                        ysb = tmps.tile(
                            [P, TQ], F32, tag="ysb", name=f"ys{qb}{tch}{cch}"
                        )
                        nc.vector.tensor_copy(ysb, yp)
                        r0 = qb * TQ + tch * P
                        nc.sync.dma_start(
                            out=y_d[r0 : r0 + P, cch * TQ : (cch + 1) * TQ],
                            in_=ysb,
                        )

            # ---- pre-roll: k-proj (heads 0,1), q(pair0, block0), vproj 0-9 ----
            for n in range(NQB):
                proj_mms(2, n, 0, KC, aux=False)
                proj_finish(2, n, aux=False)
            proj_mms(0, 0, 0, KC, aux=False)
            proj_finish(0, 0, aux=False)
            for tb in range(10):
                vproj_all(tb, aux=False)

            # ---- feed: rest of prologue in need-by order ----
            def v_item(tb):
                return (("v", tb), lambda: vproj_all(tb, aux=True))

            def p_item_a(mb, n):
                return (
                    ("pa", mb, n),
                    lambda: proj_mms(mb, n, 0, 4, aux=True),
                )

            def p_item_b(mb, n):
                def f():
                    proj_mms(mb, n, 4, 4, aux=True)
                    proj_finish(mb, n, aux=True)
                return (("p", mb, n), f)

            for tb in range(10, NKB):
                feed.append(v_item(tb))
            for n in range(NQB):
                feed.append(p_item_a(3, n))
                feed.append(p_item_b(3, n))
            feed.append(p_item_a(1, 0))
            feed.append(p_item_b(1, 0))
            for n in range(1, NQB):
                for mb in range(2):
                    feed.append(p_item_a(mb, n))
                    feed.append(p_item_b(mb, n))

            # ---- main loop ----
            for qb in range(NQB):
                if qb >= 1:
                    drain_until(("p", 0, qb))
                attention_pair(qb, 0)
                if qb >= 1:
                    finalize(qb - 1)
                drain_until(("p", 3, NQB - 1))
                drain_until(("p", 1, qb))
                attention_pair(qb, 1)
            finalize(NQB - 1)
    nc.compile()
    return nc


def _get_program():
    global _PROGRAM
    if _PROGRAM is None:
        _PROGRAM = _build_program()
    return _PROGRAM


def _eo(w):
    """[64, C] head rows -> [even(32); odd(32)]"""
    return np.concatenate([w[0::2], w[1::2]], axis=0)


def _host_prep(x, cos, sin, w_qkv, w_proj):
    f16 = np.float16
    xT = [np.ascontiguousarray(x[b].T).astype(f16) for b in range(B)]

    cosT = np.ascontiguousarray(cos.T)
    sinT = np.ascontiguousarray(sin.T)
    cc = np.tile(cosT, (4, 1)).astype(f16)
    ss = np.tile(np.concatenate([-sinT, sinT], axis=0), (2, 1)).astype(f16)
    psw = np.zeros((P, P), dtype=np.float32)
    idx = np.arange(P)
    psw[idx, idx ^ 32] = 1.0
    psw = psw.astype(f16)
    emat = np.zeros((HPC, 2 * P), dtype=np.float32)
    for p in range(2):
        for i in range(2):
            emat[2 * p + i, p * P + i * HD : p * P + (i + 1) * HD] = 1.0
    ident = np.eye(P, dtype=np.float32)
    esel = np.zeros((P, HPC), dtype=np.float32)
    for j in range(HPC):
        esel[32 * j, j] = 1.0

    wq = w_qkv[0:C]
    wk = w_qkv[C : 2 * C]
    wv = w_qkv[2 * C : 3 * C]

    in_maps = []
    for core in range(N_CORES):
        b = core // 4
        h0 = 4 * (core % 4)
        heads = [h0, h0 + 1, h0 + 2, h0 + 3]
        blocks = []
        for pair in range(2):
            ha, hb = heads[2 * pair], heads[2 * pair + 1]
            blocks.append(
                np.concatenate(
                    [_eo(wq[ha * HD : ha * HD + HD]),
                     _eo(wq[hb * HD : hb * HD + HD])],
                    axis=0,
                )
            )
        for pair in range(2):
            ha, hb = heads[2 * pair], heads[2 * pair + 1]
            blocks.append(
                np.concatenate(
                    [_eo(wk[ha * HD : ha * HD + HD]),
                     _eo(wk[hb * HD : hb * HD + HD])],
                    axis=0,
                )
            )
        wqkT = np.ascontiguousarray(
            np.concatenate(blocks, axis=0).T
        ).astype(f16)
        wvT = np.ascontiguousarray(
            wv[h0 * HD : h0 * HD + GC].T
        ).astype(f16)
        wpT = np.ascontiguousarray(
            w_proj[:, h0 * HD : h0 * HD + GC].T
        ).astype(f16)
        in_maps.append(
            {
                "xT": xT[b],
                "wqkT": wqkT,
                "wvT": wvT,
                "wpT": wpT,
                "cc": cc,
                "ss": ss,
                "psw": psw,
                "emat": emat,
                "ident": ident,
                "esel": esel,
            }
        )
    return in_maps


def kernel(x, cos, sin, mask, w_qkv, w_proj, _trace=False, _tmpdir=None):
    x = np.asarray(x, dtype=np.float32)
    cos = np.asarray(cos, dtype=np.float32)
    sin = np.asarray(sin, dtype=np.float32)
    w_qkv = np.asarray(w_qkv, dtype=np.float32)
    w_proj = np.asarray(w_proj, dtype=np.float32)
    # mask is all-ones in this problem spec: no-op in the math.

    nc = _get_program()
    in_maps = _host_prep(x, cos, sin, w_qkv, w_proj)
    res = run_bass_kernel_spmd(
        nc, in_maps, list(range(N_CORES)), trace=_trace, tmpdir=_tmpdir
    )
    out = np.empty((B, T, C), dtype=np.float32)
    for b in range(B):
        acc = res.results[4 * b]["y"].astype(np.float32).copy()
        for g in range(1, 4):
            acc += res.results[4 * b + g]["y"]
        out[b] = acc
    kernel._last_exec_time_ns = res.exec_time_ns
    return out
